# revision 44
# baseline (speedup 1.0000x reference)
"""Multi-head self-attention Trainium2 kernel (8 NeuronCores, tensor-parallel over heads).

Problem: x[2,2048,1024], W_qkv[3072,1024], b_qkv[3072], W_out[1024,1024], b_out[1024]
  qkv = x @ W_qkv.T + b_qkv ; per-head attention (16 heads, hd=64) ; out = ctx @ W_out.T + b_out
Sharding: head-parallel. Core c owns heads (2c, 2c+1) for both batches; host sums
the 8 bf16 partials and adds b_out plus the V-bias fold (W_out @ b_v).

v2 design (per-core engine budgets, TimelineSim cost model):
  - Scores run as fp8e4 DoubleRow matmuls (0.5 cycles/row): Q,K are evacuated
    from qkv psum into fp8 "strip" tiles [32, (head,ktile), n] so one DR matmul
    contracts hd=64 as 2 k-tiles of 32 partitions. Halves scores PE cost; the
    fp8 quantization error lands at ~1.4e-2 rel (budget 2e-2), measured.
  - K bias is dropped entirely: softmax over keys is invariant to the
    q . b_k term, which is constant per softmax column (exact, not approx).
    Q keeps its bias (added in the fp8 strip evac on DVE).
  - V projection is "flipped": x blocks are the PE stationary and W_v the
    moving operand, so V^T lands in psum with keys on partitions and is copied
    straight into V2 -- no V transposes at all.
  - ctx transposes for the output projection go through DMA xbar transposes
    (dma_start_transpose), freeing their psum bank and PE time.
  - PSUM banks: scores 2x[128,1024]f32 (4) + AV accum 2x[128,512] (2) +
    qkv/proj scratch (1) + V-flip accum (1) = 8.
  - The Activation engine (128 exps of [128,1024], ~133us busy) is the floor;
    PE is ~110us, DVE ~105us. One long slot stream keeps Act saturated:
    slot t emits scores(t), exp(t-1), AV(t-2); qkv/V-flip/proj work drains
    between slots under a per-slot PE cost budget with emission milestones.
"""
import sys
sys.path.insert(0, '/opt/trn_rl_repo')

import numpy as np
import ml_dtypes
from collections import deque
from contextlib import ExitStack

import concourse.bass as bass
import concourse.bacc as bacc
import concourse.tile as tile
from concourse import mybir
from concourse.bass_utils import run_bass_kernel_spmd

F32 = mybir.dt.float32
BF16 = mybir.dt.bfloat16
F8 = mybir.dt.float8e4
EXP = mybir.ActivationFunctionType.Exp
DR = mybir.MatmulPerfMode.DoubleRow
BF = ml_dtypes.bfloat16

AVLAG = 2
B, N, D = 2, 2048, 1024
BN = B * N            # 4096
HEADS, HD = 16, 64
NCORES = 8
HPC = HEADS // NCORES  # heads per core = 2
SCALE = 1.0 / np.sqrt(HD)

_cached = {}


def build_nc():
    nc = bacc.Bacc("TRN2", target_bir_lowering=False, debug=False, num_devices=NCORES)
    xT = nc.declare_dram_parameter("xT", [D, BN], BF16, isOutput=False)
    wqk = nc.declare_dram_parameter("wqk", [128, 2048], BF16, isOutput=False)
    wv = nc.declare_dram_parameter("wv", [128, 1024], BF16, isOutput=False)
    bq = nc.declare_dram_parameter("bq", [128, 1], F32, isOutput=False)
    woT = nc.declare_dram_parameter("woT", [128, D], BF16, isOutput=False)
    ident = nc.declare_dram_parameter("ident", [128, 128], F32, isOutput=False)
    out = nc.declare_dram_parameter("out", [BN, D], BF16, isOutput=True)

    with tile.TileContext(nc) as tc, ExitStack() as ctx:
        singles = ctx.enter_context(tc.tile_pool(name="singles", bufs=1))
        wq_sb = singles.tile([128, 8, 256], BF16)   # [d-part, d-tile, (q|k)x2h]
        wv_sb = singles.tile([128, 8, 128], BF16)   # [d-part, d-tile, vd 2h]
        QT8 = singles.tile([32, 4, BN], F8)         # [p32, h*2+t, n]
        KT8 = singles.tile([32, 4, BN], F8)
        V2 = singles.tile([128, 32, 130], BF16)     # per kb: [k, vd h0 | 1 | vd h1 | 1]
        woT_sb = singles.tile([128, D], BF16)
        bq_sb = singles.tile([128, 1], F32)
        id_sb = singles.tile([128, 128], F32)

        nc.gpsimd.memset(V2[:, :, 64:65], 1.0)
        nc.gpsimd.memset(V2[:, :, 129:130], 1.0)

        xpool = ctx.enter_context(tc.tile_pool(name="xg", bufs=3))

        def load_xg(g):
            # four quarter DMAs (d-half x n-half): the first compute on this
            # group can start as soon as its quarters land
            xg = xpool.tile([128, 8, 1024], BF16, name="xg")
            for nh in range(2):
                for dh in range(2):
                    src = xT[dh * 512:(dh + 1) * 512,
                             g * 1024 + nh * 512: g * 1024 + (nh + 1) * 512]
                    nc.sync.dma_start(
                        out=xg[:, dh * 4:(dh + 1) * 4, nh * 512:(nh + 1) * 512],
                        in_=src.rearrange("(d p) c -> p d c", d=4))
            return xg

        # head DMA order: critical path to scores(0) is bq+wqk+xg0[n-half 0]
        nc.sync.dma_start(out=bq_sb, in_=bq[:, :])
        nc.sync.dma_start(out=wq_sb, in_=wqk[:, :].rearrange("p (d c) -> p d c", d=8))
        xg0 = xpool.tile([128, 8, 1024], BF16, name="xg")
        for dh in range(2):
            nc.sync.dma_start(
                out=xg0[:, dh * 4:(dh + 1) * 4, 0:512],
                in_=xT[dh * 512:(dh + 1) * 512, 0:512]
                .rearrange("(d p) c -> p d c", d=4))
        nc.sync.dma_start(out=wv_sb, in_=wv[:, :].rearrange("p (d c) -> p d c", d=8))
        for dh in range(2):
            nc.sync.dma_start(
                out=xg0[:, dh * 4:(dh + 1) * 4, 512:1024],
                in_=xT[dh * 512:(dh + 1) * 512, 512:1024]
                .rearrange("(d p) c -> p d c", d=4))
        xg1 = load_xg(1)
        nc.sync.dma_start(out=woT_sb, in_=woT[:, :])
        nc.sync.dma_start(out=id_sb, in_=ident[:, :])

        with tc.tile_pool(name="pss", bufs=2, space="PSUM") as pss, \
             tc.tile_pool(name="psc", bufs=1, space="PSUM") as pscp, \
             tc.tile_pool(name="scr", bufs=1, space="PSUM") as scr, \
             tc.tile_pool(name="pvp", bufs=1, space="PSUM") as pvp, \
             tc.tile_pool(name="ep", bufs=10) as epool, \
             tc.tile_pool(name="qkbf", bufs=8) as qkpool, \
             tc.tile_pool(name="cs", bufs=8) as cspool, \
             tc.tile_pool(name="ct", bufs=10) as ctpool, \
             tc.tile_pool(name="rc", bufs=4) as rcpool, \
             tc.tile_pool(name="obp", bufs=3) as obpool:

            # p-state warmup: the PE clock ramps to full speed only after
            # ~3us of continuous execution. The head is DMA-bound anyway, so
            # run back-to-back dummy matmuls so real work starts at 2.4GHz.
            dmy = singles.tile([128, 128], BF16, name="dmy")
            nc.gpsimd.memset(dmy[:, :], 0.5)
            dmy_ps = pss.tile([128, 1024], F32, name="pS")
            for _ in range(33):
                nc.tensor.matmul(dmy_ps[:, 0:128], dmy, dmy,
                                 start=True, stop=True)
            warm = pss.tile([128, 1024], F32, name="pS")  # restore parity
            nc.tensor.matmul(warm[:, 0:128], dmy, dmy, start=True, stop=True)

            # (fn, cost_ns, is_proj): qkv blocks and V-flip groups
            scr_q = deque()
            proj_q = deque()    # proj items: no deadline, fill light slots
            blk = {"open": False}
            drained = {"n": 0}
            mile = {}           # milestone key -> required drained count
            enq = {"n": 0}

            def run_next():
                fn, _, _ = scr_q.popleft()
                fn()
                drained["n"] += 1

            def need(key):
                m = mile[key]
                while drained["n"] < m:
                    run_next()

            def enqueue(items, keys=()):
                scr_q.extend(items)
                enq["n"] += len(items)
                for k in keys:
                    mile[k] = enq["n"]

            def strip_evac(m, p, cols, mode):
                """psum [128,512] -> fp8 strips [32, j, cols]. Q (m=0) gets
                its bias; K (m=1) is a plain copy (K bias dropped: softmax
                over keys is invariant to it). mode "pool": one DVE evac to
                a bf16 staging tile, then the idle gpsimd engine fans out
                the fp8 strips (gpsimd can't read PSUM, but SBUF is fine) --
                keeps DVE off the qkv critical path. "dve"/"act": direct
                psum->fp8 strips, for the pipeline head."""
                T8 = QT8 if m == 0 else KT8
                if mode == "pool":
                    stg = qkpool.tile([128, 512], BF16, name="qkbf")
                    if m == 0:
                        nc.vector.tensor_scalar_add(stg, p, bq_sb[:, 0:1])
                    else:
                        nc.vector.tensor_copy(stg, p)
                    for j in range(4):
                        nc.gpsimd.tensor_copy(
                            T8[0:32, j, cols], stg[32 * j:32 * (j + 1), :])
                    return
                for j in range(4):
                    src = p[32 * j:32 * (j + 1), :]
                    dst = T8[0:32, j, cols]
                    if m == 0:
                        nc.vector.tensor_scalar_add(
                            dst, src, bq_sb[32 * j:32 * (j + 1), 0:1])
                    elif mode == "act":
                        nc.scalar.copy(dst, src)
                    else:
                        nc.vector.tensor_copy(dst, src)

            blkctr = {"n": 0}

            def enqueue_block(m, g, nh, xg, pool_tag=None, mode="pool"):
                """One q/k projection block: 4 scr items of 2 contraction
                tiles each (~430ns of PE per item). Blocks alternate between
                the scr and V-flip psum banks so one block's matmuls overlap
                the previous block's strip evac."""
                st = {}
                if pool_tag is None:
                    pool_tag = (scr, "scr") if blkctr["n"] % 2 == 0 \
                        else (pvp, "pv")
                blkctr["n"] += 1
                pool, tag = pool_tag

                def part(d0):
                    def fn():
                        if d0 == 0:
                            blk["open"] = pool is scr
                            st["p"] = pool.tile([128, 512], F32,
                                                tag=tag or "scr", name="fps")
                        p = st["p"]
                        for d in range(d0, d0 + 2):
                            nc.tensor.matmul(
                                p, wq_sb[:, d, m * 128:(m + 1) * 128],
                                xg[:, d, nh * 512:(nh + 1) * 512],
                                start=(d == 0), stop=(d == 7))
                        if d0 == 6:
                            cols = bass.ds(g * 1024 + nh * 512, 512)
                            strip_evac(m, p, cols, mode)
                            del st["p"]
                            blk["open"] = False
                    return fn

                enqueue([(part(d0), 430, False) for d0 in (0, 2, 4, 6)],
                        keys=[("QK"[m], g, nh)])

            def vflip_item(nb, xg):
                """V^T for keys [nb*128,(nb+1)*128): x block stationary,
                W_v moving -> psum [k,vd], then straight into V2."""
                def fn():
                    n0 = (nb % 8) * 128
                    pv = pvp.tile([128, 512], F32, tag="pv", name="pv")
                    for dt in range(8):
                        nc.tensor.matmul(
                            pv[:, 0:128], xg[:, dt, n0:n0 + 128],
                            wv_sb[:, dt, :], start=(dt == 0), stop=(dt == 7))
                    src = pv[:, 0:128].rearrange("p (h u) -> p h u", h=2)
                    dst = V2[:, nb, :].rearrange("p (t u) -> p t u", t=2)[:, :, 0:64]
                    nc.vector.tensor_copy(dst, src)
                return (fn, 430, False)

            # ---- per-chunk epilogue builders ----
            def make_post(c, psC, store):
                def fn():
                    # c=7's ctx goes out in f32: the tail transposes it on
                    # the PE (fp32 transpose) instead of the DMA xbar, whose
                    # ~2.5us/transpose latency would serialize the tail
                    cs_all = cspool.tile([128, 4, 128],
                                         F32 if c == 7 else BF16, name="cs")
                    for h in range(2):
                        rec = rcpool.tile([128, 4], F32, name="rec")
                        pC = psC[h].rearrange("p (t u) -> p t u", t=4)
                        nc.vector.reciprocal(rec, pC[:, :, 64:65])
                        nc.vector.tensor_mul(
                            cs_all[:, :, h * 64:(h + 1) * 64],
                            pC[:, :, 0:64],
                            rec.unsqueeze(2).broadcast_to([128, 4, 64]))
                    store["cs"] = cs_all
                    if c == 7:
                        return
                    # ctx transposes ride the DMA xbar; defer their emission
                    # by a slot so their cs-wait doesn't hold the SP queue
                    # (which would delay the out-stores queued behind them)
                    def trans(qb):
                        def fn2():
                            ct = ctpool.tile([128, 128], BF16, name="ct")
                            nc.sync.dma_start_transpose(
                                out=ct, in_=store["cs"][:, qb, :])
                            store[(qb, "ct")] = ct
                        return fn2
                    pending_t.setdefault(cur_t["t"] + 1, []).extend(
                        [trans(0), trans(1)])
                    pending_t.setdefault(cur_t["t"] + 2, []).extend(
                        [trans(2), trans(3)])
                return fn

            def make_projs(c, store):
                b, qB = c // 4, c % 4

                def proj_a(qb):
                    def fn():
                        po = scr.tile([128, 512], F32, tag="scr", name="po")
                        nc.tensor.matmul(po, store[(qb, "ct")], woT_sb[:, 0:512],
                                         start=True, stop=True)
                        if qb == 0:
                            store["ob"] = obpool.tile([128, 4, 1024], BF16,
                                                      name="ob")
                        nc.vector.tensor_copy(store["ob"][:, qb, 0:512], po)
                    return fn

                def proj_b(qb):
                    def fn():
                        ct = store.pop((qb, "ct"))
                        ob = store["ob"]
                        po = scr.tile([128, 512], F32, tag="scr", name="po")
                        nc.tensor.matmul(po, ct, woT_sb[:, 512:1024],
                                         start=True, stop=True)
                        nc.vector.tensor_copy(ob[:, qb, 512:1024], po)
                        if qb == 3:
                            # one combined store per chunk, dispatched from
                            # the DVE queue: DVE just wrote ob, so the store
                            # dispatch never holds a wait, and the SP queue
                            # (x-loads + ct transposes) stays short
                            rows = bass.ds(b * N + qB * 512, 512)
                            nc.sync.dma_start(
                                out=out[rows, :].rearrange(
                                    "(q p) c -> p q c", q=4),
                                in_=ob)
                    return fn

                # gate on slot >= chunk_end + 5: the DMA-xbar ct transposes
                # take ~3.5us after post; pacing a proj matmul in earlier
                # stalls the in-order PE FIFO on the transpose sem
                t0 = max(c * 16 + 22, 34 + c * 8)
                return [((proj_a(qb) if k == 0 else proj_b(qb)), 230, t0)
                        for qb in range(4) for k in range(2)]

            # ---- one flat stream of 130 global slots over 8 chunks ----
            # slot t: scores(t), exp(t-1), AV(t-2). Fillers are paced by a
            # per-slot PE cost budget so the Act engine never starves.
            psC_c = {}
            store_c = {}
            E_h = {}
            pS_h = {}
            pending_t = {}
            cur_t = {"t": 0}
            av_stash = {"fn": None}
            xgs = {0: xg0, 1: xg1}
            # head: K(0,0) into scr, Q(0,0) into the (still idle) V-flip bank
            # so Q's matmuls overlap K's strip evac; K strips ride the idle
            # Act engine.
            enqueue_block(1, 0, 0, xg0, pool_tag=(scr, "scr"), mode="act")
            enqueue_block(0, 0, 0, xg0, pool_tag=(pvp, "pv"), mode="dve")
            enqueue_block(1, 0, 1, xg0, mode="dve")
            for nb in (0, 1):
                enqueue([vflip_item(nb, xg0)], keys=[("v2", nb)])
            for nb in (2, 3, 4):
                enqueue([vflip_item(nb, xg0)], keys=[("v2", nb)])
            enqueue_block(1, 1, 0, xg1)
            for nb in (5, 6, 7):
                enqueue([vflip_item(nb, xg0)], keys=[("v2", nb)])
            enqueue_block(1, 1, 1, xg1)
            for nb in (8, 9):
                enqueue([vflip_item(nb, xg1)], keys=[("v2", nb)])
            enqueue_block(0, 0, 1, xg0)
            for nb in (10, 11, 12):
                enqueue([vflip_item(nb, xg1)], keys=[("v2", nb)])
            enqueue_block(0, 1, 0, xg1)
            for nb in (13, 14, 15):
                enqueue([vflip_item(nb, xg1)], keys=[("v2", nb)])
            enqueue_block(0, 1, 1, xg1)

            for t in range(128 + AVLAG):
                cur_t["t"] = t
                for fn2 in pending_t.pop(t, ()):
                    fn2()
                if t == 4:
                    xgs[2] = load_xg(2)
                elif t == 18:
                    xgs[3] = load_xg(3)
                    for m in (1, 0):
                        for nh in range(2):
                            enqueue_block(m, 2, nh, xgs[2])
                    for nb in range(16, 24):
                        enqueue([vflip_item(nb, xgs[2])], keys=[("v2", nb)])
                elif t == 34:
                    for m in (1, 0):
                        for nh in range(2):
                            enqueue_block(m, 3, nh, xgs[3])
                    for nb in range(24, 32):
                        enqueue([vflip_item(nb, xgs[3])], keys=[("v2", nb)])

                if t < 128:
                    c, j = t // 16, t % 16
                    b, qB = c // 4, c % 4
                    kb32 = b * 16 + j
                    need(("K", kb32 // 8, (kb32 % 8) // 4))
                    need(("Q", (b * 2048 + qB * 512) // 1024,
                          ((b * 2048 + qB * 512) % 1024) // 512))
                    qs = b * N + qB * 512
                    ks = kb32 * 128
                    pS = pss.tile([128, 1024], F32, name="pS")
                    for h in range(2):
                        nc.tensor.matmul(
                            pS[:, h * 512:(h + 1) * 512],
                            KT8[0:32, 2 * h:2 * h + 2, ks:ks + 128],
                            QT8[0:32, 2 * h:2 * h + 2, qs:qs + 512],
                            start=True, stop=True, perf_mode=DR)
                    pS_h[t] = pS
                if 1 <= t <= 128:
                    E = epool.tile([128, 1024], BF16, name="E")
                    nc.scalar.activation(E, pS_h.pop(t - 1), EXP,
                                         scale=float(SCALE))
                    E_h[t - 1] = E
                def do_av(e):
                    c, kb = e // 16, e % 16
                    b, qB = c // 4, c % 4
                    kb32 = b * 16 + kb
                    need(("v2", kb32))
                    if kb == 0:
                        psC_c[c] = (
                            pscp.tile([128, 512], F32, tag="psca", name="psCa"),
                            pscp.tile([128, 512], F32, tag="pscb", name="psCb"))
                    psC = psC_c[c]
                    Ep = E_h.pop(e)
                    # (kb0,qb0) starts the group: start=True zeroes the whole
                    # 2KB bank (incl. the other qb accumulators' columns),
                    # which is exactly the zeroing we need; the WAR edge
                    # against post(c-1)'s reads comes from the overlapping
                    # column ranges.
                    for h in range(2):
                        for qb in range(4):
                            nc.tensor.matmul(
                                psC[h][:, qb * 128:qb * 128 + 65],
                                Ep[:, h * 512 + qb * 128:
                                   h * 512 + (qb + 1) * 128],
                                V2[:, kb32, h * 65:(h + 1) * 65],
                                start=(kb == 0 and qb == 0),
                                stop=(kb == 15 and qb == 3),
                                skip_group_check=True)
                    if kb == 15:
                        store_c[c] = {}
                        make_post(c, psC, store_c[c])()
                        if c < 7:
                            proj_q.extend(make_projs(c, store_c[c]))

                # at chunk starts the AV group has a WAR on post(c-1): defer
                # its emission a full slot so the next slot's scores/exp sit
                # ahead of the stall point in the in-order PE FIFO
                if t >= AVLAG:
                    e = t - AVLAG
                    if e % 16 == 0:
                        av_stash["fn"] = (lambda e=e: do_av(e))
                    else:
                        if av_stash["fn"] is not None:
                            av_stash["fn"]()
                            av_stash["fn"] = None
                        do_av(e)

                # cost-budgeted pacing: qkv/v-flip first (they have
                # deadlines); proj items fill whatever budget remains, but
                # never inside an open qkv block (both use the scr bank)
                backlog = len(scr_q)
                budget = 800 if backlog > 16 else 540
                while scr_q:
                    fn, cost, isp = scr_q[0]
                    if cost > budget:
                        break
                    budget -= cost
                    run_next()
                while (not blk["open"] and proj_q and budget >= proj_q[0][1]
                        and t >= proj_q[0][2]):
                    fn, cost, _t0 = proj_q.popleft()
                    budget -= cost
                    fn()

            # ---- tail: chunk 7 epilogue with the now-idle Act engine ----
            for tt in sorted(pending_t):
                for fn2 in pending_t.pop(tt, ()):
                    fn2()
            while proj_q:
                proj_q.popleft()[0]()
            store = store_c[7]
            b, qB = 1, 3
            ob = obpool.tile([128, 4, 1024], BF16, name="ob")
            cs32 = store["cs"]
            # all four fp32 PE transposes fan out into the four now-free
            # psum banks, so the proj chain below streams without psum WARs
            tpools = [(scr, "scr"), (pvp, "pv"), (pscp, "psca"),
                      (pscp, "pscb")]
            cts = []
            for qb in range(4):
                pool, tag = tpools[qb]
                pt = pool.tile([128, 512], F32, tag=tag, name="pt")
                nc.tensor.matmul(pt[:, 0:128], cs32[:, qb, :], id_sb,
                                 start=True, stop=True, is_transpose=True)
                ct = ctpool.tile([128, 128], BF16, name="ct")
                nc.vector.tensor_copy(ct, pt[:, 0:128])
                cts.append(ct)
            for qb in range(4):
                po = pss.tile([128, 1024], F32, name="pS")
                nc.tensor.matmul(po[:, 0:512], cts[qb], woT_sb[:, 0:512],
                                 start=True, stop=True)
                nc.tensor.matmul(po[:, 512:1024], cts[qb], woT_sb[:, 512:1024],
                                 start=True, stop=True)
                if qb % 2 == 0:
                    nc.scalar.copy(ob[:, qb, :], po)
                else:
                    nc.vector.tensor_copy(ob[:, qb, :], po)
                rows = bass.ds(b * N + qB * 512 + qb * 128, 128)
                nc.sync.dma_start(out=out[rows, :], in_=ob[:, qb, :])
            while scr_q:
                run_next()

    nc.compile()
    return nc


def _host_prep(x, W_qkv, b_qkv, W_out):
    x2 = x.reshape(BN, D).T.astype(BF)                 # [D, BN]
    in_maps = []
    for c in range(NCORES):
        lo = HPC * c * HD                              # first ctx dim of this core
        rows = np.concatenate([np.arange(m * D + lo, m * D + lo + 128)
                               for m in range(2)])
        wqk = np.ascontiguousarray(
            W_qkv[rows, :].T.reshape(8, 128, 256).transpose(1, 0, 2)
            .reshape(128, 2048)).astype(BF)
        Wv = W_qkv[2 * D + lo: 2 * D + lo + 128, :]    # [vd, d]
        wv = np.ascontiguousarray(
            Wv.T.reshape(8, 128, 128).transpose(1, 0, 2)
            .reshape(128, 1024)).astype(BF)
        bqc = np.ascontiguousarray(
            b_qkv[lo:lo + 128].reshape(128, 1)).astype(np.float32)
        woT = np.ascontiguousarray(W_out[:, lo:lo + 128].T).astype(BF)  # [128, 1024]
        in_maps.append({
            "xT": x2, "wqk": wqk, "wv": wv, "bq": bqc, "woT": woT,
            "ident": np.eye(128, dtype=np.float32),
        })
    return in_maps


def kernel(x, W_qkv, b_qkv, W_out, b_out, _trace=False):
    x = np.asarray(x, dtype=np.float32)
    W_qkv = np.asarray(W_qkv, dtype=np.float32)
    b_qkv = np.asarray(b_qkv, dtype=np.float32)
    W_out = np.asarray(W_out, dtype=np.float32)
    b_out = np.asarray(b_out, dtype=np.float32)

    if "nc" not in _cached:
        _cached["nc"] = build_nc()
    nc = _cached["nc"]

    in_maps = _host_prep(x, W_qkv, b_qkv, W_out)
    res = run_bass_kernel_spmd(nc, in_maps, list(range(NCORES)), trace=_trace)
    _cached["last_result"] = res

    total = np.zeros((BN, D), dtype=np.float64)
    for c in range(NCORES):
        total += res.results[c]["out"].astype(np.float64)
    # V bias never went to the device: ctx bias b_v contributes the constant
    # row b_v @ W_out.T = W_out @ b_v to every output row. The K bias is
    # mathematically irrelevant (softmax-invariant).
    total += b_out.astype(np.float64)
    total += W_out.astype(np.float64) @ b_qkv[2 * D:3 * D].astype(np.float64)
    return total.reshape(B, N, D).astype(np.float32)


if __name__ == "__main__":
    rng = np.random.default_rng(0)
    x = rng.standard_normal((B, N, D), dtype=np.float32)
    s = 1.0 / np.sqrt(D)
    W_qkv = rng.uniform(-s, s, (3 * D, D)).astype(np.float32)
    b_qkv = rng.uniform(-s, s, (3 * D,)).astype(np.float32)
    W_out = rng.uniform(-s, s, (D, D)).astype(np.float32)
    b_out = rng.uniform(-s, s, (D,)).astype(np.float32)
    got = kernel(x, W_qkv, b_qkv, W_out, b_out)
    print("kernel ran, out shape", got.shape)
